# revision 2
# baseline (speedup 1.0000x reference)
"""Bidirectional cross-patch attention on 8 trn2 NeuronCores.

Sharding: data-parallel over B (4 batches x 2 cores), head-parallel within
each batch pair (6 heads per core). Each core computes q/k/v projections for
its heads, per-head masked attention, and a partial output projection; the
host sums the two partials per batch and adds the output bias.

Mask handling: allowed[i,j] = ctx_i ? ctx_j : 1. The additive -1e30 mask is
rank-1 (u_i * w_j with u=ctx, w=-1e30*(1-ctx)), so it is fused into the QK^T
matmul as a 65th contraction row. Logits are bounded (|s|~few), so softmax
needs no max subtraction: P = exp(scale*S_masked), denominator from an extra
ones-column in V.
"""

import numpy as np
import ml_dtypes

import concourse.bass as bass
import concourse.mybir as mybir
import concourse.tile as tile
from concourse.bass_utils import run_bass_kernel_spmd

BF16 = mybir.dt.bfloat16
F32 = mybir.dt.float32
bf16 = ml_dtypes.bfloat16

B, K, D, H, HD = 4, 2048, 768, 12, 64
HPC = 6        # heads per core
NPAIR = 3      # head pairs per core
NCHUNK = 6     # 768 / 128 contraction chunks
N_CORES = 8
NEG = -1e30
SCALE = 0.125  # 1/sqrt(HD)
NT = K // 128  # 16 token tiles of 128
NIB = K // 512  # 4 i-blocks of 512


def _split_multi_waits(nc, max_waits=1):
    """Walrus (CoreV3) rejects >1 sync-wait on one instruction; move extras
    onto no-op instructions inserted just before, preserving engine order."""
    for f in nc.m.functions:
        for bb in f.blocks:
            new_insts = []
            for inst in bb.instructions:
                si = inst.sync_info
                if si is not None and si.on_wait and len(si.on_wait) > max_waits:
                    waits = list(si.on_wait)
                    extra, keep = waits[:-max_waits], waits[-max_waits:]
                    for i in range(0, len(extra), max_waits):
                        chunk = extra[i:i + max_waits]
                        nop = mybir.InstNoOp(
                            name=f"waitsplit-{inst.name}-{i}",
                            engine=inst.engine,
                            sync_info=mybir.SyncInfo(on_wait=chunk, on_update=[]),
                        )
                        new_insts.append(nop)
                    si.on_wait = keep
                new_insts.append(inst)
            bb.instructions[:] = new_insts


def _build_nc():
    nc = bass.Bass()
    xT = nc.dram_tensor("xT", [NCHUNK, 128, K], BF16, kind="ExternalInput")
    wqT = nc.dram_tensor("wqT", [NCHUNK, 128, 384], BF16, kind="ExternalInput")
    wkT = nc.dram_tensor("wkT", [NCHUNK, 128, 384], BF16, kind="ExternalInput")
    wvT = nc.dram_tensor("wvT", [NCHUNK, 128, 384], BF16, kind="ExternalInput")
    woT = nc.dram_tensor("woT", [NPAIR, 128, D], BF16, kind="ExternalInput")
    bqv = nc.dram_tensor("bqv", [128, NPAIR], F32, kind="ExternalInput")
    bkv = nc.dram_tensor("bkv", [128, NPAIR], F32, kind="ExternalInput")
    bvv = nc.dram_tensor("bvv", [1, 384], BF16, kind="ExternalInput")
    uv = nc.dram_tensor("uv", [1, K], BF16, kind="ExternalInput")
    wv = nc.dram_tensor("wv", [1, K], BF16, kind="ExternalInput")
    out = nc.dram_tensor("out", [K, D], F32, kind="ExternalOutput")

    with tile.TileContext(nc) as tc:
        with (
            tc.tile_pool(name="const", bufs=1) as constp,
            tc.tile_pool(name="qpair", bufs=2) as qpp,
            tc.tile_pool(name="heads", bufs=2) as qkh,
            tc.tile_pool(name="vpool", bufs=2) as vpl,
            tc.tile_pool(name="ptp", bufs=16) as ptp,
            tc.tile_pool(name="yhp", bufs=2) as yhp,
            tc.tile_pool(name="ypk", bufs=1) as ypp,
            tc.tile_pool(name="small", bufs=2) as smp,
            tc.tile_pool(name="ost", bufs=2) as osp,
            tc.tile_pool(name="dscr", bufs=4, space="DRAM") as dsp,
            tc.tile_pool(name="ps_s", bufs=1, space="PSUM") as ps_s,
            tc.tile_pool(name="ps_y", bufs=2, space="PSUM") as ps_y,
            tc.tile_pool(name="ps_p", bufs=2, space="PSUM") as ps_p,
        ):
            # ---- load persistent operands
            xts = []
            for c in range(NCHUNK):
                t = constp.tile([128, K], BF16, tag=f"xt{c}")
                nc.sync.dma_start(out=t, in_=xT[c])
                xts.append(t)

            def load_w(name, dram, n, cols):
                ts = []
                for c in range(n):
                    t = constp.tile([128, cols], BF16, tag=f"{name}{c}")
                    nc.sync.dma_start(out=t, in_=dram[c])
                    ts.append(t)
                return ts

            wqs = load_w("wq", wqT, NCHUNK, 384)
            wks = load_w("wk", wkT, NCHUNK, 384)
            wvs = load_w("wv", wvT, NCHUNK, 384)
            wos = load_w("wo", woT, NPAIR, D)

            bq_sb = constp.tile([128, NPAIR], F32, tag="bq")
            nc.sync.dma_start(out=bq_sb, in_=bqv[:])
            bk_sb = constp.tile([128, NPAIR], F32, tag="bk")
            nc.sync.dma_start(out=bk_sb, in_=bkv[:])
            bv_sb = constp.tile([1, 384], BF16, tag="bv")
            nc.sync.dma_start(out=bv_sb, in_=bvv[:])
            ones_sb = constp.tile([1, 128], BF16, tag="ones")
            nc.vector.memset(ones_sb, 1.0)

            ypk = [
                ypp.tile([128, K], BF16, tag=f"ypk{c}", name=f"ypk{c}")
                for c in range(NPAIR)
            ]

            for p in range(NPAIR):
                hsl = slice(p * 128, (p + 1) * 128)
                # ---- q^T / k^T projection for this head pair: [128(dq), K]
                pair_tiles = {}
                for nm, ws, b_sb in (("q", wqs, bq_sb), ("k", wks, bk_sb)):
                    tp = qpp.tile([128, K], BF16, tag=f"{nm}pair")
                    for ib in range(NIB):
                        isl = slice(ib * 512, (ib + 1) * 512)
                        ps = ps_p.tile([128, 512], F32, tag="proj")
                        for c in range(NCHUNK):
                            nc.tensor.matmul(
                                ps, ws[c][:, hsl], xts[c][:, isl],
                                start=(c == 0), stop=(c == NCHUNK - 1),
                            )
                        nc.vector.tensor_scalar_add(tp[:, isl], ps, b_sb[:, p:p + 1])
                    pair_tiles[nm] = tp
                # per-head 65-row tiles: rows 0..63 head data, row 64 = mask row
                qh, kh = [], []
                for hh in range(2):
                    qt = qkh.tile([65, K], BF16, tag=f"qh{hh}")
                    kt = qkh.tile([65, K], BF16, tag=f"kh{hh}")
                    nc.sync.dma_start(out=qt[0:64, :], in_=pair_tiles["q"][hh * 64:(hh + 1) * 64, :])
                    nc.sync.dma_start(out=kt[0:64, :], in_=pair_tiles["k"][hh * 64:(hh + 1) * 64, :])
                    nc.sync.dma_start(out=qt[64:65, :], in_=uv[:])
                    nc.sync.dma_start(out=kt[64:65, :], in_=wv[:])
                    qh.append(qt)
                    kh.append(kt)
                # ---- v projection: natural layout [t, dv], packed per (tile, head)
                vh = vpl.tile([128, NT, 2, 65], BF16, tag="vh")
                nc.vector.memset(vh[:, :, :, 64:65], 1.0)
                for tt in range(NT):
                    tsl = slice(tt * 128, (tt + 1) * 128)
                    ps = ps_p.tile([128, 512], F32, tag="proj")
                    for c in range(NCHUNK):
                        nc.tensor.matmul(
                            ps[:, 0:128], xts[c][:, tsl], wvs[c][:, hsl],
                            start=(c == 0), stop=False,
                        )
                    nc.tensor.matmul(
                        ps[:, 0:128], ones_sb[:, 0:128], bv_sb[:, hsl],
                        start=False, stop=True,
                    )
                    for hh in range(2):
                        nc.vector.tensor_copy(vh[:, tt, hh, 0:64], ps[:, hh * 64:(hh + 1) * 64])

                # ---- attention per head
                for hh in range(2):
                    pts = []
                    for jc in range(NT):
                        s_ps = ps_s.tile([128, 2048], F32, tag="s")
                        for ib in range(NIB):
                            isl = slice(ib * 512, (ib + 1) * 512)
                            nc.tensor.matmul(
                                s_ps[:, isl],
                                kh[hh][:, jc * 128:(jc + 1) * 128],
                                qh[hh][:, isl],
                                start=True, stop=True,
                            )
                        ptile = ptp.tile([128, K], BF16, tag="pt")
                        nc.scalar.activation(
                            ptile, s_ps, mybir.ActivationFunctionType.Exp, scale=SCALE
                        )
                        pts.append(ptile)
                    yht = yhp.tile([64, K], BF16, tag="yh")
                    for ib in range(NIB):
                        isl = slice(ib * 512, (ib + 1) * 512)
                        y_ps = ps_y.tile([65, 512], F32, tag="y")
                        for jc in range(NT):
                            nc.tensor.matmul(
                                y_ps, vh[:, jc, hh, :], pts[jc][:, isl],
                                start=(jc == 0), stop=(jc == NT - 1),
                            )
                        # normalize: r = 1/sum row; broadcast via DRAM bounce
                        rt = smp.tile([65, 512], F32, tag="r")
                        nc.vector.reciprocal(rt[64:65, :], y_ps[64:65, :])
                        dscr = dsp.tile([1, 512], F32, tag="ds")
                        nc.sync.dma_start(out=dscr, in_=rt[64:65, :])
                        rb = smp.tile([64, 512], F32, tag="rb")
                        bcast = bass.AP(
                            tensor=dscr.tensor, offset=dscr.offset,
                            ap=[[0, 64]] + [list(dscr.ap[-1])],
                        )
                        nc.sync.dma_start(out=rb, in_=bcast)
                        nc.vector.tensor_mul(yht[:, isl], y_ps[0:64, :], rb)
                    # repack into [128(dy), K] via DMA (partition shift)
                    nc.sync.dma_start(out=ypk[p][hh * 64:(hh + 1) * 64, :], in_=yht)

            # ---- output projection (partial over this core's 384 dy)
            for tt in range(NT):
                tsl = slice(tt * 128, (tt + 1) * 128)
                ot = osp.tile([128, D], F32, tag="ost")
                for oc, osz in ((0, 512), (1, 256)):
                    ps = ps_p.tile([128, 512], F32, tag="proj")
                    for c in range(NPAIR):
                        nc.tensor.matmul(
                            ps[:, 0:osz],
                            ypk[c][:, tsl],
                            wos[c][:, oc * 512:oc * 512 + osz],
                            start=(c == 0), stop=(c == NPAIR - 1),
                        )
                    nc.vector.tensor_copy(ot[:, oc * 512:oc * 512 + osz], ps[:, 0:osz])
                nc.sync.dma_start(out=out[tsl, :], in_=ot)

    _split_multi_waits(nc)
    return nc


_NC = None


def _get_nc():
    global _NC
    if _NC is None:
        _NC = _build_nc()
    return _NC


def make_in_maps(x, is_context, Wq, bq, Wk, bk, Wv, bv, Wo):
    """Host-side sharding/layout prep: per-core input dicts."""
    in_maps = []
    xTb = {}
    uvb = {}
    wvb = {}
    for b in range(B):
        ctx = is_context[b].astype(np.float32)
        xTb[b] = np.ascontiguousarray(x[b].T).astype(bf16).reshape(NCHUNK, 128, K)
        uvb[b] = ctx.reshape(1, K).astype(bf16)
        wvb[b] = (NEG * (1.0 - ctx)).reshape(1, K).astype(bf16)
    WoT = np.ascontiguousarray(Wo.T)  # [dy, dout]
    for core in range(N_CORES):
        b = core // 2
        half = core % 2
        sel = slice(384 * half, 384 * (half + 1))
        in_maps.append({
            "xT": xTb[b],
            "wqT": np.ascontiguousarray(Wq[sel].T).astype(bf16).reshape(NCHUNK, 128, 384),
            "wkT": np.ascontiguousarray(Wk[sel].T).astype(bf16).reshape(NCHUNK, 128, 384),
            "wvT": np.ascontiguousarray(Wv[sel].T).astype(bf16).reshape(NCHUNK, 128, 384),
            "woT": WoT[sel].astype(bf16).reshape(NPAIR, 128, D),
            "bqv": np.ascontiguousarray(bq[sel].reshape(NPAIR, 128).T).astype(np.float32),
            "bkv": np.ascontiguousarray(bk[sel].reshape(NPAIR, 128).T).astype(np.float32),
            "bvv": bv[sel].reshape(1, 384).astype(bf16),
            "uv": uvb[b],
            "wv": wvb[b],
        })
    return in_maps


def combine_results(results, bo):
    out = np.zeros((B, K, D), np.float32)
    for b in range(B):
        out[b] = results[2 * b]["out"] + results[2 * b + 1]["out"] + bo
    return out


def kernel(x, is_context, Wq, bq, Wk, bk, Wv, bv, Wo, bo):
    x = np.asarray(x, np.float32)
    is_context = np.asarray(is_context)
    args = [np.asarray(a, np.float32) for a in (Wq, bq, Wk, bk, Wv, bv, Wo)]
    nc = _get_nc()
    in_maps = make_in_maps(x, is_context, *args)
    res = run_bass_kernel_spmd(nc, in_maps, list(range(N_CORES)), trace=False)
    return combine_results(res.results, np.asarray(bo, np.float32))
